# revision 1
# baseline (speedup 1.0000x reference)
"""Distributed Trainium2 kernel for nn_AttentionBsl (LN -> QKV -> 16-head
attention -> output projection) on 8 NeuronCores.

Sharding: token-parallel. Core j handles batch j//4, tokens
[512*(j%4), 512*(j%4+1)).  Each core layernorms its token slice, computes
Q/K/V projections for its tokens (all heads), AllGathers K and V^T within
its 4-core batch group, runs attention for its 512 queries against all
2048 keys, and applies the full output projection for its token slice.
Output shards are disjoint slices of the final output -> host concat.

Matmuls run as float32r (FP32 storage, ~FP22 compute) at full PE rate;
the attn@V stage runs bf16 (free dim 65 would be 4x-penalized in f32r).
Softmax skips max-subtraction (scores ~ N(0,1), exp is safe in f32), and
row sums come free from a ones-augmented 65th V column.
"""

import sys

if "/opt/trn_rl_repo" not in sys.path:
    sys.path.insert(0, "/opt/trn_rl_repo")

import numpy as np

DIM = 1024
SEQ = 2048
BATCH = 2
HEADS = 16
DH = 64
NCORES = 8
GROUP = 4          # cores per batch group
NT = SEQ // GROUP  # 512 tokens per core
P = 128
CT = DIM // P      # 8 contraction tiles
NPAIR = HEADS // 2  # 8 head pairs
KT = SEQ // P      # 16 key tiles
QT = NT // P       # 4 query tiles
EPS = 1e-5

_CACHE = {}


def _build(debug=False):
    import concourse.bass as bass  # noqa: F401
    import concourse.mybir as mybir
    import concourse.tile as tile
    from concourse import bacc
    from concourse.masks import make_identity

    f32 = mybir.dt.float32
    f32r = mybir.dt.float32r
    bf16 = mybir.dt.bfloat16
    AF = mybir.ActivationFunctionType
    ALU = mybir.AluOpType

    nc = bacc.Bacc("TRN2", target_bir_lowering=False, debug=False,
                   num_devices=NCORES)

    x_in = nc.dram_tensor("x", [DIM, NT], f32, kind="ExternalInput")
    wq_in = nc.dram_tensor("wq", [DIM, DIM], f32r, kind="ExternalInput")
    wk_in = nc.dram_tensor("wk", [DIM, DIM], f32r, kind="ExternalInput")
    wv_in = nc.dram_tensor("wv", [DIM, DIM], f32r, kind="ExternalInput")
    wo_in = nc.dram_tensor("wo", [DIM, DIM], f32r, kind="ExternalInput")
    g_in = nc.dram_tensor("gamma", [1, DIM], f32, kind="ExternalInput")
    b_in = nc.dram_tensor("beta", [1, DIM], f32, kind="ExternalInput")
    out_ext = nc.dram_tensor("out", [DIM, NT], f32, kind="ExternalOutput")
    if debug:
        dbg_h = nc.dram_tensor("dbg_h", [DIM, NT], f32, kind="ExternalOutput")
        dbg_q = nc.dram_tensor("dbg_q", [DIM, NT], f32, kind="ExternalOutput")
        dbg_k = nc.dram_tensor("dbg_k", [P, SEQ], f32, kind="ExternalOutput")
        dbg_v = nc.dram_tensor("dbg_v", [P, HEADS * (DH + 1)], f32,
                               kind="ExternalOutput")
        dbg_ao = nc.dram_tensor("dbg_ao", [DIM, NT], f32, kind="ExternalOutput")

    # AllGather bounce buffers. Row layout of cc_in:
    #   rows [0, 1024):     K for this core's tokens, [o_row, token] f32
    #   rows [1024, 1536):  V^T, [token, (head, d)] bf16 viewed as 512 f32
    CC_ROWS = DIM + NT
    cc_in = nc.dram_tensor("cc_in", [CC_ROWS, NT], f32)
    cc_out = nc.dram_tensor("cc_out", [GROUP * CC_ROWS, NT], f32)

    with tile.TileContext(nc) as tc:
        with (
            tc.tile_pool(name="const", bufs=1) as constp,
            tc.tile_pool(name="qp", bufs=CT) as qpool,
            tc.tile_pool(name="aop", bufs=CT) as aopool,
        ):
            # ---- constants ----
            ident = constp.tile([P, P], f32)
            make_identity(nc, ident[:])
            grow = constp.tile([1, DIM], f32)
            nc.sync.dma_start(out=grow[:], in_=g_in[:])
            brow = constp.tile([1, DIM], f32)
            nc.sync.dma_start(out=brow[:], in_=b_in[:])
            ones_row = constp.tile([1, NT], f32)
            nc.vector.memset(ones_row[:], 1.0)
            ones_col = constp.tile([P, 1], f32)
            nc.vector.memset(ones_col[:], 1.0)

            # =========================================================
            # Phase A: LayerNorm + K/V^T projections + AllGather + Q.
            # =========================================================
            with (
                tc.tile_pool(name="hp_", bufs=CT) as hpool,
                tc.tile_pool(name="wp", bufs=CT) as wpool,
                tc.tile_pool(name="stage", bufs=3) as stagep,
            ):
              with (
                tc.tile_pool(name="xp", bufs=CT) as xp,
                tc.tile_pool(name="stats", bufs=8) as statp,
                tc.tile_pool(name="lnps", bufs=2, space="PSUM") as lnps,
                tc.tile_pool(name="abps", bufs=2, space="PSUM") as abps,
              ):
                x_t = []
                for ci in range(CT):
                    t = xp.tile([P, NT], f32, tag="x", name="x_t")
                    nc.sync.dma_start(out=t[:],
                                      in_=x_in[ci * P:(ci + 1) * P, :])
                    x_t.append(t)

                # ---- LN stats via ones-matmul partition reductions ----
                ps_sum = lnps.tile([1, NT], f32)
                ps_sq = lnps.tile([1, NT], f32)
                for ci in range(CT):
                    sq = xp.tile([P, NT], f32, tag="sq", name="sq_t", bufs=3)
                    nc.scalar.activation(sq[:], x_t[ci][:], AF.Square)
                    nc.tensor.matmul(ps_sum[:], ones_col[:], x_t[ci][:],
                                     start=(ci == 0), stop=(ci == CT - 1))
                    nc.tensor.matmul(ps_sq[:], ones_col[:], sq[:],
                                     start=(ci == 0), stop=(ci == CT - 1))

                mean = statp.tile([1, NT], f32, tag="st", name="mean")
                nc.vector.tensor_scalar_mul(mean[:], ps_sum[:], 1.0 / DIM)
                var = statp.tile([1, NT], f32, tag="st", name="var")
                nc.vector.tensor_scalar_mul(var[:], ps_sq[:], 1.0 / DIM)
                m2 = statp.tile([1, NT], f32, tag="st", name="m2")
                nc.vector.tensor_tensor(m2[:], mean[:], mean[:], ALU.mult)
                nc.vector.tensor_tensor(var[:], var[:], m2[:], ALU.subtract)
                nc.vector.tensor_scalar_add(var[:], var[:], EPS)
                rv = statp.tile([1, NT], f32, tag="st", name="rv")
                nc.vector.reciprocal(rv[:], var[:])
                rstd = statp.tile([1, NT], f32, tag="st", name="rstd")
                nc.scalar.activation(rstd[:], rv[:], AF.Sqrt)
                nb = statp.tile([1, NT], f32, tag="st", name="nb")
                nc.vector.tensor_tensor(nb[:], mean[:], rstd[:], ALU.mult)
                nc.vector.tensor_scalar_mul(nb[:], nb[:], -1.0)

                # h = x * (gamma x rstd) + (gamma x nb + beta x 1)
                h_t = []
                for ci in range(CT):
                    ab = abps.tile([P, 2 * NT], f32, tag="ab", name="ab")
                    gsl = grow[0:1, ci * P:(ci + 1) * P]
                    bsl = brow[0:1, ci * P:(ci + 1) * P]
                    nc.tensor.matmul(ab[:, 0:NT], gsl, rstd[:],
                                     start=True, stop=True)
                    nc.tensor.matmul(ab[:, NT:2 * NT], gsl, nb[:],
                                     start=True, stop=False)
                    nc.tensor.matmul(ab[:, NT:2 * NT], bsl, ones_row[:],
                                     start=False, stop=True)
                    ht = hpool.tile([P, NT], f32r, tag="h", name="h_t")
                    nc.vector.tensor_tensor(ht[:], x_t[ci][:], ab[:, 0:NT],
                                            ALU.mult)
                    nc.vector.tensor_tensor(ht[:], ht[:], ab[:, NT:2 * NT],
                                            ALU.add)
                    h_t.append(ht)

              with (
                tc.tile_pool(name="pjps", bufs=2, space="PSUM") as pjps,
              ):
                # ---- K projection -> cc_in ----
                wk_t = []
                for ci in range(CT):
                    t = wpool.tile([P, DIM], f32r, tag="w", name="wk_t")
                    nc.sync.dma_start(out=t[:],
                                      in_=wk_in[ci * P:(ci + 1) * P, :])
                    wk_t.append(t)
                for ot in range(CT):
                    ps = pjps.tile([P, NT], f32, tag="pj", name="kps", bufs=4)
                    for ci in range(CT):
                        nc.tensor.matmul(
                            ps[:],
                            wk_t[ci][:, ot * P:(ot + 1) * P],
                            h_t[ci][:],
                            start=(ci == 0), stop=(ci == CT - 1))
                    st = stagep.tile([P, NT], f32, tag="stg", name="kstg")
                    nc.vector.tensor_copy(st[:], ps[:])
                    nc.sync.dma_start(out=cc_in[ot * P:(ot + 1) * P, :],
                                      in_=st[:])

                # ---- V^T projection -> cc_in (bf16) ----
                wv_t = []
                for ci in range(CT):
                    t = wpool.tile([P, DIM], f32r, tag="w", name="wv_t")
                    nc.sync.dma_start(out=t[:],
                                      in_=wv_in[ci * P:(ci + 1) * P, :])
                    wv_t.append(t)
                for tt in range(QT):
                    ps = pjps.tile([P, DIM], f32, tag="vpj", name="vps", bufs=2)
                    for ci in range(CT):
                        lh = h_t[ci][:, tt * P:(tt + 1) * P]
                        nc.tensor.matmul(
                            ps[:, 0:512], lh,
                            wv_t[ci][:, 0:512],
                            start=(ci == 0), stop=(ci == CT - 1))
                        nc.tensor.matmul(
                            ps[:, 512:1024], lh,
                            wv_t[ci][:, 512:1024],
                            start=(ci == 0), stop=(ci == CT - 1))
                    st = stagep.tile([P, DIM], bf16, tag="stg", name="vstg")
                    nc.vector.tensor_copy(st[:], ps[:])
                    nc.sync.dma_start(
                        out=cc_in[DIM + tt * P:DIM + (tt + 1) * P, :],
                        in_=st[:].bitcast(f32))

                nc.gpsimd.collective_compute(
                    "AllGather", ALU.bypass,
                    replica_groups=[[0, 1, 2, 3], [4, 5, 6, 7]],
                    ins=[cc_in.ap().opt()],
                    outs=[cc_out.ap().opt()],
                )

                # ---- Q projection (overlaps the AllGather) ----
                wq_t = []
                for ci in range(CT):
                    t = wpool.tile([P, DIM], f32r, tag="w", name="wq_t")
                    nc.sync.dma_start(out=t[:],
                                      in_=wq_in[ci * P:(ci + 1) * P, :])
                    wq_t.append(t)
                q_t = []
                for ot in range(CT):
                    ps = pjps.tile([P, NT], f32, tag="pj", name="qps", bufs=4)
                    for ci in range(CT):
                        nc.tensor.matmul(
                            ps[:],
                            wq_t[ci][:, ot * P:(ot + 1) * P],
                            h_t[ci][:],
                            start=(ci == 0), stop=(ci == CT - 1))
                    qt_ = qpool.tile([P, NT], f32r, tag="q", name="q_t")
                    nc.vector.tensor_copy(qt_[:], ps[:])
                    q_t.append(qt_)
                if debug:
                    for ci in range(CT):
                        nc.sync.dma_start(
                            out=dbg_h[ci * P:(ci + 1) * P, :],
                            in_=h_t[ci][:].bitcast(f32))
                        nc.sync.dma_start(
                            out=dbg_q[ci * P:(ci + 1) * P, :],
                            in_=q_t[ci][:].bitcast(f32))

            # =========================================================
            # Phase B: attention.
            #   scoresT[k, q] = K^T q  (two row-tiled f32r matmuls)
            #   attnT = exp(scoresT / 8)            (ScalarE, bf16 out)
            #   out[q, d(+sum)] += attnT^T @ V_aug  (bf16 matmuls)
            # =========================================================
            with (
                tc.tile_pool(name="kgp", bufs=NPAIR) as kgp,
                tc.tile_pool(name="vgp", bufs=KT) as vgp,
                tc.tile_pool(name="attnT", bufs=KT + 2) as atp,
                tc.tile_pool(name="pairsb", bufs=6) as pairp,
                tc.tile_pool(name="rcp", bufs=8) as rcp,
                tc.tile_pool(name="scps", bufs=2, space="PSUM") as scps,
                tc.tile_pool(name="avps", bufs=2, space="PSUM") as avps,
                tc.tile_pool(name="tpps", bufs=2, space="PSUM") as tpps,
            ):
                k_g = []
                for hp in range(NPAIR):
                    t = kgp.tile([P, SEQ], f32r, tag="kg", name="k_g")
                    for r in range(GROUP):
                        nc.sync.dma_start(
                            out=t[:, r * NT:(r + 1) * NT],
                            in_=cc_out[r * CC_ROWS + hp * P:
                                       r * CC_ROWS + (hp + 1) * P,
                                       :].bitcast(f32r))
                    k_g.append(t)
                v_g = []
                for kt in range(KT):
                    t = vgp.tile([P, HEADS, DH + 1], bf16, tag="vg",
                                 name="v_g")
                    nc.vector.memset(t[:, :, DH:DH + 1], 1.0)
                    r = kt // (KT // GROUP)
                    lt = kt % (KT // GROUP)
                    src = cc_out[r * CC_ROWS + DIM + lt * P:
                                 r * CC_ROWS + DIM + (lt + 1) * P, :]
                    nc.sync.dma_start(
                        out=t[:, :, 0:DH],
                        in_=src.bitcast(bf16).rearrange(
                            "p (h d) -> p h d", h=HEADS))
                    v_g.append(t)
                if debug:
                    nc.sync.dma_start(out=dbg_k[:],
                                      in_=k_g[0][:].bitcast(f32))
                    vdbg = atp.tile([P, HEADS * (DH + 1)], f32, tag="vd",
                                    name="vdbg", bufs=1)
                    nc.vector.tensor_copy(
                        vdbg[:], v_g[0][:].rearrange("p h d -> p (h d)"))
                    nc.sync.dma_start(out=dbg_v[:], in_=vdbg[:])

                ao_t = [aopool.tile([P, NT], f32r, tag="ao", name="ao")
                        for _ in range(CT)]
                for hp in range(NPAIR):
                    av = [avps.tile([P, QT, 72], f32, tag="av", name="av")
                          for _ in range(2)]
                    at_c = []
                    for kt in range(KT):
                        sc = scps.tile([P, 2 * NT], f32, tag="sc", name="sc")
                        ksl = k_g[hp]
                        nc.tensor.matmul(
                            sc[:, 0:NT],
                            ksl[0:DH, kt * P:(kt + 1) * P],
                            q_t[hp][0:DH, :],
                            start=True, stop=True)
                        nc.tensor.matmul(
                            sc[:, NT:2 * NT],
                            ksl[DH:P, kt * P:(kt + 1) * P],
                            q_t[hp][DH:P, :],
                            start=True, stop=True)
                        at = atp.tile([P, 2 * NT], bf16, tag="at", name="at")
                        nc.scalar.activation(at[:], sc[:], AF.Exp,
                                             scale=float(1.0 / np.sqrt(DH)))
                        at_c.append(at)
                    # One PSUM accumulation group at a time per bank:
                    # start=True clears has_written for the WHOLE bank, so
                    # groups sharing a bank must not interleave.
                    for hi in range(2):
                        hg = hp * 2 + hi
                        for qt in range(QT):
                            for kt in range(KT):
                                nc.tensor.matmul(
                                    av[hi][:, qt, 0:DH + 1],
                                    at_c[kt][:, hi * NT + qt * P:
                                             hi * NT + (qt + 1) * P],
                                    v_g[kt][:, hg, :],
                                    start=(kt == 0), stop=(kt == KT - 1))
                    # normalize + transpose into ao rows
                    for qt in range(QT):
                        pair_sb = pairp.tile([P, P], f32, tag="pr",
                                             name="pair_sb")
                        for hi in range(2):
                            rc = rcp.tile([P, 1], f32, tag="rc", name="rc")
                            nc.vector.reciprocal(rc[:],
                                                 av[hi][:, qt, DH:DH + 1])
                            nc.vector.tensor_scalar(
                                pair_sb[:, hi * DH:(hi + 1) * DH],
                                av[hi][:, qt, 0:DH],
                                rc[:], None, ALU.mult)
                        tp = tpps.tile([P, P], f32, tag="tp", name="tp")
                        nc.tensor.transpose(tp[:], pair_sb[:], ident[:])
                        nc.vector.tensor_copy(
                            ao_t[hp][:, qt * P:(qt + 1) * P], tp[:])

            # =========================================================
            # Phase C: output projection for this token slice.
            # =========================================================
            if debug:
                for ci in range(CT):
                    nc.sync.dma_start(out=dbg_ao[ci * P:(ci + 1) * P, :],
                                      in_=ao_t[ci][:].bitcast(f32))
            with (
                tc.tile_pool(name="wop", bufs=CT) as wop,
                tc.tile_pool(name="outsb", bufs=3) as outp,
                tc.tile_pool(name="ops", bufs=4, space="PSUM") as ops,
            ):
                wo_t = []
                for ci in range(CT):
                    t = wop.tile([P, DIM], f32r, tag="w", name="wo_t")
                    nc.sync.dma_start(out=t[:],
                                      in_=wo_in[ci * P:(ci + 1) * P, :])
                    wo_t.append(t)
                for ot in range(CT):
                    ps = ops.tile([P, NT], f32, tag="o", name="ops_t")
                    for ci in range(CT):
                        nc.tensor.matmul(
                            ps[:],
                            wo_t[ci][:, ot * P:(ot + 1) * P],
                            ao_t[ci][:],
                            start=(ci == 0), stop=(ci == CT - 1))
                    ost = outp.tile([P, NT], f32, tag="ou", name="ost")
                    nc.vector.tensor_copy(ost[:], ps[:])
                    nc.sync.dma_start(out=out_ext[ot * P:(ot + 1) * P, :],
                                      in_=ost[:])

    nc.compile()
    return nc


def _get_nc(debug=False):
    key = ("nc", debug)
    if key not in _CACHE:
        _CACHE[key] = _build(debug)
    return _CACHE[key]


def kernel(x, w_qkv, w_out, ln_gamma, ln_beta, _profile=False, _debug=False):
    from concourse.bass_utils import run_bass_kernel_spmd

    x = np.asarray(x, np.float32)
    w_qkv = np.asarray(w_qkv, np.float32)
    w_out = np.asarray(w_out, np.float32)
    ln_gamma = np.asarray(ln_gamma, np.float32)
    ln_beta = np.asarray(ln_beta, np.float32)

    wq = np.ascontiguousarray(w_qkv[0:DIM].T)
    wk = np.ascontiguousarray(w_qkv[DIM:2 * DIM].T)
    wv = np.ascontiguousarray(w_qkv[2 * DIM:3 * DIM].T)
    wo = np.ascontiguousarray(w_out.T)
    grow = np.ascontiguousarray(ln_gamma.reshape(1, DIM))
    brow = np.ascontiguousarray(ln_beta.reshape(1, DIM))

    in_maps = []
    for j in range(NCORES):
        b, c = divmod(j, GROUP)
        in_maps.append({
            "x": np.ascontiguousarray(x[:, c * NT:(c + 1) * NT, b]),
            "wq": wq, "wk": wk, "wv": wv, "wo": wo,
            "gamma": grow, "beta": brow,
        })

    nc = _get_nc(_debug)
    res = run_bass_kernel_spmd(nc, in_maps, core_ids=list(range(NCORES)),
                               trace=_profile)
    if _profile:
        _CACHE["last_result"] = res

    out = np.empty((DIM, SEQ, BATCH), np.float32)
    for j in range(NCORES):
        b, c = divmod(j, GROUP)
        out[:, c * NT:(c + 1) * NT, b] = res.results[j]["out"]
    if _debug:
        _CACHE["dbg"] = res.results
    return out



# revision 19
# speedup vs baseline: 1.1955x; 1.1955x over previous
"""Distributed Trainium2 kernel for nn_AttentionBsl (LN -> QKV -> 16-head
attention -> output projection) on 8 NeuronCores.

Sharding: token-parallel. Core j handles batch j//4, tokens
[512*(j%4), 512*(j%4+1)).  Each core layernorms its token slice, computes
K/V/Q projections for its tokens, AllGathers K and V (bf16, in four
pipelined chunks of 512KB so gathers overlap compute), runs attention for
its 512 queries against all 2048 keys, and applies the output projection.

v2 changes vs v1:
- LN stats/affine matmuls run f32r (v1 used f32 at 1/4 PE rate).
- K/V payloads cast to bf16; the collective is split into 4 chunks
  (K-heads0-7, V-heads0-7, K-heads8-15, V-heads8-15) fired as soon as
  their producers finish, and a dummy AllGather at t=0 absorbs the
  cross-core rendezvous barrier.
- Weights stream through SBUF in per-ci 4KB chunks (ci-outer projection
  loops holding 8 PSUM banks), so projections start as soon as the first
  chunk lands and SBUF never holds a full weight matrix.
- attn@V restructured: stationary = V-tile [128k, 65] (65th column of
  ones yields softmax denominators on partition 64), moving = all 512
  queries.  Fewer weight loads, no PE transposes.
- exp is pipelined 2 k-tiles behind the score matmuls so neither PE nor
  the activation engine stalls (v1 ping-ponged, resetting PE pstate).
"""

import sys

if "/opt/trn_rl_repo" not in sys.path:
    sys.path.insert(0, "/opt/trn_rl_repo")

import numpy as np

DIM = 1024
SEQ = 2048
BATCH = 2
HEADS = 16
DH = 64
NCORES = 8
GROUP = 4          # cores per batch group
NT = SEQ // GROUP  # 512 tokens per core
P = 128
CT = DIM // P      # 8 contraction tiles
NPAIR = HEADS // 2  # 8 head pairs
KT = SEQ // P      # 16 key tiles
EPS = 1e-5
LAG = 2            # attnV trails scores by LAG k-tiles

_CACHE = {}


def _build(debug=False):
    import concourse.bass as bass  # noqa: F401
    import concourse.mybir as mybir
    import concourse.tile as tile
    from concourse import bacc

    f32 = mybir.dt.float32
    f32r = mybir.dt.float32r
    bf16 = mybir.dt.bfloat16
    AF = mybir.ActivationFunctionType
    ALU = mybir.AluOpType
    RG = [[0, 1, 2, 3], [4, 5, 6, 7]]

    nc = bacc.Bacc("TRN2", target_bir_lowering=False, debug=False,
                   num_devices=NCORES)

    x_in = nc.dram_tensor("x", [DIM, NT], f32, kind="ExternalInput")
    wq_in = nc.dram_tensor("wq", [DIM, DIM], f32r, kind="ExternalInput")
    wk_in = nc.dram_tensor("wk", [DIM, DIM], f32r, kind="ExternalInput")
    wv_in = nc.dram_tensor("wv", [DIM, DIM], f32r, kind="ExternalInput")
    wo_in = nc.dram_tensor("wo", [DIM, DIM], f32r, kind="ExternalInput")
    g_in = nc.dram_tensor("gamma", [1, DIM], f32, kind="ExternalInput")
    b_in = nc.dram_tensor("beta", [1, DIM], f32, kind="ExternalInput")
    out_ext = nc.dram_tensor("out", [DIM, NT], f32, kind="ExternalOutput")
    if debug:
        dbg_h = nc.dram_tensor("dbg_h", [DIM, NT], f32, kind="ExternalOutput")
        dbg_q = nc.dram_tensor("dbg_q", [DIM, NT], f32, kind="ExternalOutput")
        dbg_k = nc.dram_tensor("dbg_k", [P, SEQ], f32, kind="ExternalOutput")
        dbg_v = nc.dram_tensor("dbg_v", [P, HEADS * (DH + 1)], f32,
                               kind="ExternalOutput")
        dbg_ao = nc.dram_tensor("dbg_ao", [DIM, NT], f32, kind="ExternalOutput")

    # Collective bounce buffers.  K stored [o_row, token] bf16, viewed as
    # f32 pairs along token; V stored [token, (head d)] bf16.  Four 512KB
    # chunks: K heads 0-7 / V heads 0-7 / K heads 8-15 / V heads 8-15.
    cc_dummy_in = nc.dram_tensor("ccdin", [4, 1], f32)
    cc_dummy_out = nc.dram_tensor("ccdout", [16, 1], f32)
    cc_in = []
    cc_out = []
    for name in ("k1", "v1", "k2", "v2"):
        cc_in.append(nc.dram_tensor(f"cc{name}i", [4 * P, NT // 2], f32))
        cc_out.append(nc.dram_tensor(f"cc{name}o", [GROUP * 4 * P, NT // 2],
                                     f32))
    cck_in = [cc_in[0], cc_in[2]]
    cck_out = [cc_out[0], cc_out[2]]
    ccv_in = [cc_in[1], cc_in[3]]
    ccv_out = [cc_out[1], cc_out[3]]

    with tile.TileContext(nc) as tc:
        with (
            tc.tile_pool(name="const", bufs=1) as constp,
            tc.tile_pool(name="qp", bufs=NPAIR) as qpool,
            tc.tile_pool(name="kgp", bufs=NPAIR) as kgp,
            tc.tile_pool(name="vgp", bufs=KT) as vgp,
            tc.tile_pool(name="aop", bufs=CT) as aopool,
        ):
            # Rendezvous absorber: fire a tiny collective immediately.
            nc.gpsimd.collective_compute(
                "AllGather", ALU.bypass, replica_groups=RG,
                ins=[cc_dummy_in.ap().opt()], outs=[cc_dummy_out.ap().opt()])

            # ---- constants ----
            # gamma/beta as per-partition columns: gcol[p, ci] = gamma[ci*P+p]
            gcol = constp.tile([P, CT], f32)
            nc.sync.dma_start(
                out=gcol[:],
                in_=g_in.ap().rearrange("o (c p) -> p c o", p=P)[:, :, 0])
            bcol = constp.tile([P, CT], f32)
            nc.sync.dma_start(
                out=bcol[:],
                in_=b_in.ap().rearrange("o (c p) -> p c o", p=P)[:, :, 0])
            ones_col = constp.tile([P, 1], bf16)
            nc.gpsimd.memset(ones_col[:], 1.0)
            ones_rowp = constp.tile([1, P], f32)
            nc.gpsimd.memset(ones_rowp[:], 1.0)
            ones_row64 = constp.tile([1, DH], bf16)
            nc.gpsimd.memset(ones_row64[:], 1.0)

            # =========================================================
            # Phase A: LayerNorm -> K/V projections (-> AllGather) -> Q.
            # =========================================================
            with (
                tc.tile_pool(name="xh", bufs=1) as xhp,
                tc.tile_pool(name="wp", bufs=3) as wpool,
                tc.tile_pool(name="stage", bufs=1) as stagep,
            ):
                x_t = xhp.tile([P, CT, NT], f32, name="x_t")
                for ci in range(CT):
                    nc.sync.dma_start(out=x_t[:, ci, :],
                                      in_=x_in[ci * P:(ci + 1) * P, :])
                h_t = xhp.tile([P, CT, NT], f32r, name="h_t")

                with (
                    tc.tile_pool(name="sqp", bufs=3) as sqp,
                    tc.tile_pool(name="stats", bufs=1) as statp,
                    tc.tile_pool(name="tup", bufs=3) as tup,
                    tc.tile_pool(name="lnps", bufs=2, space="PSUM") as lnps,
                    tc.tile_pool(name="abps", bufs=1, space="PSUM") as abps,
                ):
                    ps_sum = lnps.tile([1, NT], f32)
                    ps_sq = lnps.tile([1, NT], f32)
                    for ci in range(CT):
                        xb = sqp.tile([P, NT], bf16, tag="xb", name="xb_t")
                        nc.scalar.activation(xb[:], x_t[:, ci, :], AF.Copy)
                        sq = sqp.tile([P, NT], bf16, tag="sq", name="sq_t")
                        nc.scalar.activation(sq[:], x_t[:, ci, :], AF.Square)
                        nc.tensor.matmul(ps_sum[:], ones_col[:], xb[:],
                                         start=(ci == 0), stop=(ci == CT - 1))
                        nc.tensor.matmul(ps_sq[:], ones_col[:], sq[:],
                                         start=(ci == 0), stop=(ci == CT - 1))

                    mean = statp.tile([1, NT], f32, tag="st", name="mean", bufs=6)
                    nc.vector.tensor_scalar_mul(mean[:], ps_sum[:], 1.0 / DIM)
                    var = statp.tile([1, NT], f32, tag="st", name="var", bufs=6)
                    nc.vector.tensor_scalar_mul(var[:], ps_sq[:], 1.0 / DIM)
                    m2 = statp.tile([1, NT], f32, tag="st", name="m2", bufs=6)
                    nc.vector.tensor_tensor(m2[:], mean[:], mean[:], ALU.mult)
                    nc.vector.tensor_tensor(var[:], var[:], m2[:],
                                            ALU.subtract)
                    nc.vector.tensor_scalar_add(var[:], var[:], EPS)
                    rv = statp.tile([1, NT], f32, tag="st", name="rv", bufs=6)
                    nc.vector.reciprocal(rv[:], var[:])
                    rstd = statp.tile([1, NT], f32, tag="st", name="rstd", bufs=6)
                    nc.scalar.activation(rstd[:], rv[:], AF.Sqrt)
                    nb = statp.tile([1, NT], f32, tag="st", name="nb", bufs=6)
                    nc.vector.tensor_tensor(nb[:], mean[:], rstd[:], ALU.mult)
                    nc.vector.tensor_scalar_mul(nb[:], nb[:], -1.0)

                    # Broadcast rows: bc_r = 1 x rstd, bc_m = 1 x (-mean*rstd)
                    ab = abps.tile([P, 2 * NT], f32, tag="ab", name="ab")
                    nc.tensor.matmul(ab[:, 0:NT], ones_rowp[:], rstd[:],
                                     start=True, stop=True)
                    nc.tensor.matmul(ab[:, NT:2 * NT], ones_rowp[:], nb[:],
                                     start=True, stop=True)
                    # h = gamma*(x*bc_r) + (gamma*bc_m + beta)
                    for ci in range(CT):
                        gsl = gcol[:, ci:ci + 1]
                        bsl = bcol[:, ci:ci + 1]
                        tt = tup.tile([P, NT], f32, tag="tu", name="t_t")
                        nc.vector.scalar_tensor_tensor(
                            tt[:], x_t[:, ci, :], gsl, ab[:, 0:NT],
                            ALU.mult, ALU.mult)
                        ut = tup.tile([P, NT], f32, tag="tu", name="u_t")
                        nc.vector.tensor_scalar(
                            ut[:], ab[:, NT:2 * NT], gsl, bsl,
                            ALU.mult, ALU.add)
                        nc.vector.tensor_tensor(h_t[:, ci, :], tt[:], ut[:],
                                                ALU.add)

                # ---- K projection (ci-outer, weights streamed) ----
                with tc.tile_pool(name="kjps", bufs=CT, space="PSUM") as kjps:
                    kps = [kjps.tile([P, NT], f32, tag="pj", name="kps")
                           for _ in range(CT)]
                    for ci in range(CT):
                        wc = wpool.tile([P, DIM], f32r, tag="w", name="wk_c")
                        nc.sync.dma_start(out=wc[:],
                                          in_=wk_in[ci * P:(ci + 1) * P, :])
                        for ot in range(CT):
                            nc.tensor.matmul(
                                kps[ot][:], wc[:, ot * P:(ot + 1) * P],
                                h_t[:, ci, :],
                                start=(ci == 0), stop=(ci == CT - 1))
                    kstage = [stagep.tile([P, CT // 2, NT], bf16, tag="kst",
                                          name=f"kstage{i}", bufs=2)
                              for i in range(2)]
                    for ot in range(CT):
                        nc.scalar.activation(kstage[ot // 4][:, ot % 4, :],
                                             kps[ot][:], AF.Copy)
                        if ot == 3 or ot == CT - 1:
                            half = ot // 4
                            nc.sync.dma_start(
                                out=cck_in[half].ap().bitcast(bf16).rearrange(
                                    "(c p) n -> p c n", p=P),
                                in_=kstage[half][:])
                            nc.gpsimd.collective_compute(
                                "AllGather", ALU.bypass, replica_groups=RG,
                                ins=[cck_in[half].ap().opt()],
                                outs=[cck_out[half].ap().opt()])

                # ---- V projection (h-stationary, ci-outer) ----
                with tc.tile_pool(name="vjps", bufs=GROUP,
                                  space="PSUM") as vjps:
                    vps = [vjps.tile([P, DIM], f32, tag="vpj", name="vps")
                           for _ in range(GROUP)]
                    for ci in range(CT):
                        wc = wpool.tile([P, DIM], f32r, tag="w", name="wv_c")
                        nc.sync.dma_start(out=wc[:],
                                          in_=wv_in[ci * P:(ci + 1) * P, :])
                        for tt in range(GROUP):
                            for hf in range(2):
                                nc.tensor.matmul(
                                    vps[tt][:, hf * NT:(hf + 1) * NT],
                                    h_t[:, ci, tt * P:(tt + 1) * P],
                                    wc[:, hf * NT:(hf + 1) * NT],
                                    start=(ci == 0), stop=(ci == CT - 1))
                    vstage = stagep.tile([P, GROUP, DIM], bf16, tag="vst",
                                         name="vstage", bufs=1)
                    for tt in range(GROUP):
                        nc.vector.tensor_copy(vstage[:, tt, :], vps[tt][:])
                    for half in range(2):
                        nc.sync.dma_start(
                            out=ccv_in[half].ap().bitcast(bf16).rearrange(
                                "(t p) o -> p t o", p=P),
                            in_=vstage[:, :, half * NT:(half + 1) * NT])
                        nc.gpsimd.collective_compute(
                            "AllGather", ALU.bypass, replica_groups=RG,
                            ins=[ccv_in[half].ap().opt()],
                            outs=[ccv_out[half].ap().opt()])

                # ---- Q projection (ci-outer) ----
                with tc.tile_pool(name="qjps", bufs=CT, space="PSUM") as qjps:
                    qps = [qjps.tile([P, NT], f32, tag="qj", name="qps")
                           for _ in range(CT)]
                    for ci in range(CT):
                        wc = wpool.tile([P, DIM], f32r, tag="w", name="wq_c")
                        nc.sync.dma_start(out=wc[:],
                                          in_=wq_in[ci * P:(ci + 1) * P, :])
                        for ot in range(CT):
                            nc.tensor.matmul(
                                qps[ot][:], wc[:, ot * P:(ot + 1) * P],
                                h_t[:, ci, :],
                                start=(ci == 0), stop=(ci == CT - 1))
                    q_t = []
                    for ot in range(CT):
                        qt_ = qpool.tile([P, NT], bf16, tag="q", name="q_t")
                        nc.scalar.activation(qt_[:], qps[ot][:], AF.Copy)
                        q_t.append(qt_)

            # ---- gathered K/V into SBUF ----
            k_g = []
            for hp in range(NPAIR):
                t = kgp.tile([P, GROUP, NT], bf16, tag="kg", name="k_g")
                src = cck_out[hp // 4].ap().bitcast(bf16).rearrange(
                    "(r c p) n -> p c r n", r=GROUP, p=P)
                nc.sync.dma_start(out=t[:], in_=src[:, hp % 4, :, :])
                k_g.append(t)
            v_g = []
            for kt in range(KT):
                t = vgp.tile([P, HEADS, DH + 1], bf16, tag="vg", name="v_g")
                nc.gpsimd.memset(t[:, :, DH:DH + 1], 1.0)
                r, tt = divmod(kt, GROUP)
                for half in range(2):
                    src = ccv_out[half].ap().bitcast(bf16).rearrange(
                        "(r t p) (h d) -> p r t h d", r=GROUP, t=GROUP, h=8)
                    nc.sync.dma_start(
                        out=t[:, half * 8:(half + 1) * 8, 0:DH],
                        in_=src[:, r, tt, :, :])
                v_g.append(t)


            # =========================================================
            # Phase B: attention.
            #   scT[k, q] = K^T q  (per pair, two 64-contraction matmuls)
            #   at = exp(scT / 8)          (ScalarE, bf16)
            #   av[d(+den), q] += V_aug^T @ at   (v-stationary matmuls)
            #   ao[hd, q] = av[d, q] * recip(den)[q]
            # =========================================================
            ao_t = [aopool.tile([P, NT], f32r, tag="ao", name="ao")
                    for _ in range(CT)]
            with (
                tc.tile_pool(name="atp", bufs=8) as atp,
                tc.tile_pool(name="rcp", bufs=1) as rcp,
                tc.tile_pool(name="tmpb", bufs=2) as tmpbp,
                tc.tile_pool(name="scps", bufs=2, space="PSUM") as scps,
                tc.tile_pool(name="avps", bufs=3, space="PSUM") as avps,
                tc.tile_pool(name="bcps", bufs=1, space="PSUM") as bcps,
            ):
                prev = None  # (hp, av_pair) awaiting normalize

                def normalize(hp, av_pair):
                    for hi in range(2):
                        rc = rcp.tile([1, NT], bf16, tag="rc", name="rc", bufs=4)
                        with nc.allow_low_precision(
                                reason="fp22 recip of softmax denom is ample"):
                            nc.vector.reciprocal(rc[:],
                                                 av_pair[hi][DH:DH + 1, :])
                        bc = bcps.tile([DH, NT], f32, tag="bc", name="bc")
                        nc.tensor.matmul(bc[:], ones_row64[:],
                                         rc[:],
                                         start=True, stop=True)
                        bcs = tmpbp.tile([DH, NT], f32, tag="bcs",
                                         name="bcs", bufs=2)
                        nc.vector.tensor_copy(bcs[:], bc[:])
                        if hi == 0:
                            nc.vector.tensor_tensor(
                                ao_t[hp][0:DH, :], av_pair[0][0:DH, :],
                                bcs[:], ALU.mult)
                        else:
                            tmpb = tmpbp.tile([DH, NT], f32r, tag="tb",
                                              name="tmpb")
                            nc.vector.tensor_tensor(
                                tmpb[:], av_pair[1][0:DH, :], bcs[:],
                                ALU.mult)
                            nc.sync.dma_start(out=ao_t[hp][DH:P, :],
                                              in_=tmpb[:])

                for hp in range(NPAIR):
                    av = [avps.tile([DH + 1, NT], f32, tag="av", name="av")
                          for _ in range(2)]
                    if prev is not None:
                        normalize(*prev)
                    prev = (hp, av)
                    at_c = []

                    def attnv(kt, av=av, at_c=at_c, hp=hp):
                        for hi in range(2):
                            nc.tensor.matmul(
                                av[hi][:],
                                v_g[kt][:, hp * 2 + hi, :],
                                at_c[kt][:, hi * NT:(hi + 1) * NT],
                                start=(kt == 0), stop=(kt == KT - 1))

                    for kt in range(KT):
                        sc = scps.tile([P, 2 * NT], f32, tag="sc", name="sc")
                        r, tt = divmod(kt, GROUP)
                        ksl = k_g[hp][:, r, tt * P:(tt + 1) * P]
                        nc.tensor.matmul(
                            sc[:, 0:NT],
                            ksl[0:DH, :],
                            q_t[hp][0:DH, :], start=True, stop=True)
                        nc.tensor.matmul(
                            sc[:, NT:2 * NT],
                            ksl[DH:P, :],
                            q_t[hp][DH:P, :], start=True, stop=True)
                        at = atp.tile([P, 2 * NT], bf16, tag="at", name="at")
                        nc.scalar.activation(at[:], sc[:], AF.Exp,
                                             scale=float(1.0 / np.sqrt(DH)))
                        at_c.append(at)
                        if kt >= LAG:
                            attnv(kt - LAG)
                    for kt in range(KT - LAG, KT):
                        attnv(kt)
                normalize(*prev)

            if debug:
                for ci in range(CT):
                    nc.sync.dma_start(out=dbg_ao[ci * P:(ci + 1) * P, :],
                                      in_=ao_t[ci][:])

            # =========================================================
            # Phase C: output projection (wo streamed).
            # =========================================================
            with (
                tc.tile_pool(name="wop", bufs=3) as wop,
                tc.tile_pool(name="outsb", bufs=2) as outp,
                tc.tile_pool(name="ops", bufs=CT, space="PSUM") as ops,
            ):
                ops_t = [ops.tile([P, NT], f32, tag="o", name="ops_t")
                         for _ in range(CT)]
                for ci in range(CT):
                    wc = wop.tile([P, DIM], f32r, tag="wo", name="wo_c")
                    nc.sync.dma_start(out=wc[:],
                                      in_=wo_in[ci * P:(ci + 1) * P, :])
                    for ot in range(CT):
                        nc.tensor.matmul(
                            ops_t[ot][:], wc[:, ot * P:(ot + 1) * P],
                            ao_t[ci][:],
                            start=(ci == 0), stop=(ci == CT - 1))
                for ot in range(CT):
                    ost = outp.tile([P, NT], f32, tag="ou", name="ost")
                    nc.vector.tensor_copy(ost[:], ops_t[ot][:])
                    nc.sync.dma_start(out=out_ext[ot * P:(ot + 1) * P, :],
                                      in_=ost[:])

    nc.compile()
    return nc


def _get_nc(debug=False):
    key = ("nc", debug)
    if key not in _CACHE:
        _CACHE[key] = _build(debug)
    return _CACHE[key]


def kernel(x, w_qkv, w_out, ln_gamma, ln_beta, _profile=False, _debug=False):
    from concourse.bass_utils import run_bass_kernel_spmd

    x = np.asarray(x, np.float32)
    w_qkv = np.asarray(w_qkv, np.float32)
    w_out = np.asarray(w_out, np.float32)
    ln_gamma = np.asarray(ln_gamma, np.float32)
    ln_beta = np.asarray(ln_beta, np.float32)

    wq = np.ascontiguousarray(w_qkv[0:DIM].T)
    wk = np.ascontiguousarray(w_qkv[DIM:2 * DIM].T)
    wv = np.ascontiguousarray(w_qkv[2 * DIM:3 * DIM].T)
    wo = np.ascontiguousarray(w_out.T)
    grow = np.ascontiguousarray(ln_gamma.reshape(1, DIM))
    brow = np.ascontiguousarray(ln_beta.reshape(1, DIM))

    in_maps = []
    for j in range(NCORES):
        b, c = divmod(j, GROUP)
        in_maps.append({
            "x": np.ascontiguousarray(x[:, c * NT:(c + 1) * NT, b]),
            "wq": wq, "wk": wk, "wv": wv, "wo": wo,
            "gamma": grow, "beta": brow,
        })

    nc = _get_nc(_debug)
    res = run_bass_kernel_spmd(nc, in_maps, core_ids=list(range(NCORES)),
                               trace=_profile)
    if _profile:
        _CACHE["last_result"] = res

    out = np.empty((DIM, SEQ, BATCH), np.float32)
    for j in range(NCORES):
        b, c = divmod(j, GROUP)
        out[:, c * NT:(c + 1) * NT, b] = res.results[j]["out"]
    if _debug:
        _CACHE["dbg"] = res.results
    return out


# revision 25
# speedup vs baseline: 1.2293x; 1.0283x over previous
"""Distributed Trainium2 kernel for nn_AttentionBsl (LN -> QKV -> 16-head
attention -> output projection) on 8 NeuronCores.

Sharding: token-parallel. Core j handles batch j//4, tokens
[512*(j%4), 512*(j%4+1)).  Each core layernorms its token slice, computes
K/V/Q projections for its tokens, AllGathers K and V (bf16, in four
pipelined chunks of 512KB so gathers overlap compute), runs attention for
its 512 queries against all 2048 keys, and applies the output projection.

v2 changes vs v1:
- LN stats/affine matmuls run f32r (v1 used f32 at 1/4 PE rate).
- K/V payloads cast to bf16; the collective is split into 4 chunks
  (K-heads0-7, V-heads0-7, K-heads8-15, V-heads8-15) fired as soon as
  their producers finish, and a dummy AllGather at t=0 absorbs the
  cross-core rendezvous barrier.
- Weights stream through SBUF in per-ci 4KB chunks (ci-outer projection
  loops holding 8 PSUM banks), so projections start as soon as the first
  chunk lands and SBUF never holds a full weight matrix.
- attn@V restructured: stationary = V-tile [128k, 65] (65th column of
  ones yields softmax denominators on partition 64), moving = all 512
  queries.  Fewer weight loads, no PE transposes.
- exp is pipelined 2 k-tiles behind the score matmuls so neither PE nor
  the activation engine stalls (v1 ping-ponged, resetting PE pstate).
"""

import sys

if "/opt/trn_rl_repo" not in sys.path:
    sys.path.insert(0, "/opt/trn_rl_repo")

import numpy as np

DIM = 1024
SEQ = 2048
BATCH = 2
HEADS = 16
DH = 64
NCORES = 8
GROUP = 4          # cores per batch group
NT = SEQ // GROUP  # 512 tokens per core
P = 128
CT = DIM // P      # 8 contraction tiles
NPAIR = HEADS // 2  # 8 head pairs
KT = SEQ // P      # 16 key tiles
EPS = 1e-5
LAG = 2            # attnV trails scores by LAG k-tiles

_CACHE = {}


def _build(debug=False):
    import concourse.bass as bass  # noqa: F401
    import concourse.mybir as mybir
    import concourse.tile as tile
    from concourse import bacc

    f32 = mybir.dt.float32
    f32r = mybir.dt.float32r
    bf16 = mybir.dt.bfloat16
    AF = mybir.ActivationFunctionType
    ALU = mybir.AluOpType
    RG = [[0, 1, 2, 3], [4, 5, 6, 7]]

    nc = bacc.Bacc("TRN2", target_bir_lowering=False, debug=False,
                   num_devices=NCORES)

    x_in = nc.dram_tensor("x", [DIM, NT], f32, kind="ExternalInput")
    wq_in = nc.dram_tensor("wq", [DIM, DIM], f32r, kind="ExternalInput")
    wk_in = nc.dram_tensor("wk", [DIM, DIM], f32r, kind="ExternalInput")
    wv_in = nc.dram_tensor("wv", [DIM, DIM], f32r, kind="ExternalInput")
    wo_in = nc.dram_tensor("wo", [DIM, DIM], f32r, kind="ExternalInput")
    g_in = nc.dram_tensor("gamma", [1, DIM], f32, kind="ExternalInput")
    b_in = nc.dram_tensor("beta", [1, DIM], f32, kind="ExternalInput")
    out_ext = nc.dram_tensor("out", [DIM, NT], f32, kind="ExternalOutput")
    if debug:
        dbg_h = nc.dram_tensor("dbg_h", [DIM, NT], f32, kind="ExternalOutput")
        dbg_q = nc.dram_tensor("dbg_q", [DIM, NT], f32, kind="ExternalOutput")
        dbg_k = nc.dram_tensor("dbg_k", [P, SEQ], f32, kind="ExternalOutput")
        dbg_v = nc.dram_tensor("dbg_v", [P, HEADS * (DH + 1)], f32,
                               kind="ExternalOutput")
        dbg_ao = nc.dram_tensor("dbg_ao", [DIM, NT], f32, kind="ExternalOutput")

    # Collective bounce buffers.  K stored [o_row, token] bf16, viewed as
    # f32 pairs along token; V stored [token, (head d)] bf16.  Four 512KB
    # chunks: K heads 0-7 / V heads 0-7 / K heads 8-15 / V heads 8-15.
    cc_dummy_in = nc.dram_tensor("ccdin", [4, 1], f32)
    cc_dummy_out = nc.dram_tensor("ccdout", [16, 1], f32)
    cc_in = []
    cc_out = []
    for name in ("k1", "v1", "k2", "v2"):
        cc_in.append(nc.dram_tensor(f"cc{name}i", [4 * P, NT // 2], f32))
        cc_out.append(nc.dram_tensor(f"cc{name}o", [GROUP * 4 * P, NT // 2],
                                     f32))
    cck_in = [cc_in[0], cc_in[2]]
    cck_out = [cc_out[0], cc_out[2]]
    ccv_in = [cc_in[1], cc_in[3]]
    ccv_out = [cc_out[1], cc_out[3]]

    with tile.TileContext(nc) as tc:
        with (
            tc.tile_pool(name="const", bufs=1) as constp,
            tc.tile_pool(name="qp", bufs=NPAIR) as qpool,
            tc.tile_pool(name="kgp", bufs=NPAIR) as kgp,
            tc.tile_pool(name="vgp", bufs=KT) as vgp,
            tc.tile_pool(name="aop", bufs=CT) as aopool,
        ):
            # Rendezvous absorber: fire a tiny collective immediately.
            nc.gpsimd.collective_compute(
                "AllGather", ALU.bypass, replica_groups=RG,
                ins=[cc_dummy_in.ap().opt()], outs=[cc_dummy_out.ap().opt()])

            # ---- constants ----
            # gamma/beta as per-partition columns: gcol[p, ci] = gamma[ci*P+p]
            gcol = constp.tile([P, CT], f32)
            nc.sync.dma_start(
                out=gcol[:],
                in_=g_in.ap().rearrange("o (c p) -> p c o", p=P)[:, :, 0])
            bcol = constp.tile([P, CT], f32)
            nc.sync.dma_start(
                out=bcol[:],
                in_=b_in.ap().rearrange("o (c p) -> p c o", p=P)[:, :, 0])
            ones_col = constp.tile([P, 1], bf16)
            nc.gpsimd.memset(ones_col[:], 1.0)
            ones_rowp = constp.tile([1, P], f32)
            nc.gpsimd.memset(ones_rowp[:], 1.0)
            ones_row64 = constp.tile([1, DH], bf16)
            nc.gpsimd.memset(ones_row64[:], 1.0)

            # =========================================================
            # Phase A: LayerNorm -> K/V projections (-> AllGather) -> Q.
            # =========================================================
            with (
                tc.tile_pool(name="xh", bufs=1) as xhp,
                tc.tile_pool(name="wp", bufs=3) as wpool,
                tc.tile_pool(name="stage", bufs=1) as stagep,
            ):
                x_t = xhp.tile([P, CT, NT], f32, name="x_t")
                for ci in range(CT):
                    nc.sync.dma_start(out=x_t[:, ci, :],
                                      in_=x_in[ci * P:(ci + 1) * P, :])
                h_t = xhp.tile([P, CT, NT], f32r, name="h_t")

                with (
                    tc.tile_pool(name="sqp", bufs=3) as sqp,
                    tc.tile_pool(name="stats", bufs=1) as statp,
                    tc.tile_pool(name="tup", bufs=3) as tup,
                    tc.tile_pool(name="lnps", bufs=2, space="PSUM") as lnps,
                    tc.tile_pool(name="abps", bufs=1, space="PSUM") as abps,
                ):
                    ps_sum = lnps.tile([1, NT], f32)
                    ps_sq = lnps.tile([1, NT], f32)
                    for ci in range(CT):
                        xb = sqp.tile([P, NT], bf16, tag="xb", name="xb_t")
                        nc.vector.tensor_copy(xb[:], x_t[:, ci, :])
                        sq = sqp.tile([P, NT], bf16, tag="sq", name="sq_t")
                        nc.scalar.activation(sq[:], x_t[:, ci, :], AF.Square)
                        nc.tensor.matmul(ps_sum[:], ones_col[:], xb[:],
                                         start=(ci == 0), stop=(ci == CT - 1))
                        nc.tensor.matmul(ps_sq[:], ones_col[:], sq[:],
                                         start=(ci == 0), stop=(ci == CT - 1))

                    mean = statp.tile([1, NT], f32, tag="st", name="mean", bufs=6)
                    nc.vector.tensor_scalar_mul(mean[:], ps_sum[:], 1.0 / DIM)
                    var = statp.tile([1, NT], f32, tag="st", name="var", bufs=6)
                    nc.vector.tensor_scalar_mul(var[:], ps_sq[:], 1.0 / DIM)
                    m2 = statp.tile([1, NT], f32, tag="st", name="m2", bufs=6)
                    nc.vector.tensor_tensor(m2[:], mean[:], mean[:], ALU.mult)
                    nc.vector.tensor_tensor(var[:], var[:], m2[:],
                                            ALU.subtract)
                    nc.vector.tensor_scalar_add(var[:], var[:], EPS)
                    rv = statp.tile([1, NT], f32, tag="st", name="rv", bufs=6)
                    nc.vector.reciprocal(rv[:], var[:])
                    rstd = statp.tile([1, NT], f32, tag="st", name="rstd", bufs=6)
                    nc.scalar.activation(rstd[:], rv[:], AF.Sqrt)
                    nb = statp.tile([1, NT], f32, tag="st", name="nb", bufs=6)
                    nc.vector.tensor_tensor(nb[:], mean[:], rstd[:], ALU.mult)
                    nc.vector.tensor_scalar_mul(nb[:], nb[:], -1.0)

                    # Broadcast rows: bc_r = 1 x rstd, bc_m = 1 x (-mean*rstd)
                    ab = abps.tile([P, 2 * NT], f32, tag="ab", name="ab")
                    nc.tensor.matmul(ab[:, 0:NT], ones_rowp[:], rstd[:],
                                     start=True, stop=True)
                    nc.tensor.matmul(ab[:, NT:2 * NT], ones_rowp[:], nb[:],
                                     start=True, stop=True)
                    # h = gamma*(x*bc_r) + (gamma*bc_m + beta)
                    for ci in range(CT):
                        gsl = gcol[:, ci:ci + 1]
                        bsl = bcol[:, ci:ci + 1]
                        tt = tup.tile([P, NT], f32, tag="tu", name="t_t")
                        nc.vector.scalar_tensor_tensor(
                            tt[:], x_t[:, ci, :], gsl, ab[:, 0:NT],
                            ALU.mult, ALU.mult)
                        ut = tup.tile([P, NT], f32, tag="tu", name="u_t")
                        nc.vector.tensor_scalar(
                            ut[:], ab[:, NT:2 * NT], gsl, bsl,
                            ALU.mult, ALU.add)
                        nc.vector.tensor_tensor(h_t[:, ci, :], tt[:], ut[:],
                                                ALU.add)

                # ---- K projection (ci-outer, weights streamed) ----
                with tc.tile_pool(name="kjps", bufs=CT, space="PSUM") as kjps:
                    kps = [kjps.tile([P, NT], f32, tag="pj", name="kps")
                           for _ in range(CT)]
                    for ci in range(CT):
                        wc = wpool.tile([P, DIM], f32r, tag="w", name="wk_c")
                        nc.sync.dma_start(out=wc[:],
                                          in_=wk_in[ci * P:(ci + 1) * P, :])
                        for ot in range(CT):
                            nc.tensor.matmul(
                                kps[ot][:], wc[:, ot * P:(ot + 1) * P],
                                h_t[:, ci, :],
                                start=(ci == 0), stop=(ci == CT - 1))
                    kstage = [stagep.tile([P, CT // 2, NT], bf16, tag="kst",
                                          name=f"kstage{i}", bufs=2)
                              for i in range(2)]
                    for ot in range(CT):
                        nc.scalar.activation(kstage[ot // 4][:, ot % 4, :],
                                             kps[ot][:], AF.Copy)
                        if ot == 3 or ot == CT - 1:
                            half = ot // 4
                            nc.sync.dma_start(
                                out=cck_in[half].ap().bitcast(bf16).rearrange(
                                    "(c p) n -> p c n", p=P),
                                in_=kstage[half][:])
                    # Fire K heads 0-7 now; K2/V1/V2 are ordered below.
                    nc.gpsimd.collective_compute(
                        "AllGather", ALU.bypass, replica_groups=RG,
                        ins=[cck_in[0].ap().opt()],
                        outs=[cck_out[0].ap().opt()])

                # ---- V projection (h-stationary, ci-outer) ----
                with tc.tile_pool(name="vjps", bufs=GROUP,
                                  space="PSUM") as vjps:
                    vps = [vjps.tile([P, DIM], f32, tag="vpj", name="vps")
                           for _ in range(GROUP)]
                    for ci in range(CT):
                        wc = wpool.tile([P, DIM], f32r, tag="w", name="wv_c")
                        nc.sync.dma_start(out=wc[:],
                                          in_=wv_in[ci * P:(ci + 1) * P, :])
                        for tt in range(GROUP):
                            for hf in range(2):
                                nc.tensor.matmul(
                                    vps[tt][:, hf * NT:(hf + 1) * NT],
                                    h_t[:, ci, tt * P:(tt + 1) * P],
                                    wc[:, hf * NT:(hf + 1) * NT],
                                    start=(ci == 0), stop=(ci == CT - 1))
                    vstage = stagep.tile([P, GROUP, DIM], bf16, tag="vst",
                                         name="vstage", bufs=1)
                    for tt in range(GROUP):
                        nc.vector.tensor_copy(vstage[:, tt, :], vps[tt][:])
                    for half in range(2):
                        nc.sync.dma_start(
                            out=ccv_in[half].ap().bitcast(bf16).rearrange(
                                "(t p) o -> p t o", p=P),
                            in_=vstage[:, :, half * NT:(half + 1) * NT])
                    # Chain order: K1 (above), V1, K2, V2 — matches the
                    # order phase B consumes them (pairs 0-3 then 4-7).
                    nc.gpsimd.collective_compute(
                        "AllGather", ALU.bypass, replica_groups=RG,
                        ins=[ccv_in[0].ap().opt()],
                        outs=[ccv_out[0].ap().opt()])
                    nc.gpsimd.collective_compute(
                        "AllGather", ALU.bypass, replica_groups=RG,
                        ins=[cck_in[1].ap().opt()],
                        outs=[cck_out[1].ap().opt()])
                    nc.gpsimd.collective_compute(
                        "AllGather", ALU.bypass, replica_groups=RG,
                        ins=[ccv_in[1].ap().opt()],
                        outs=[ccv_out[1].ap().opt()])

                # ---- Q projection (ci-outer) ----
                with tc.tile_pool(name="qjps", bufs=CT, space="PSUM") as qjps:
                    qps = [qjps.tile([P, NT], f32, tag="qj", name="qps")
                           for _ in range(CT)]
                    for ci in range(CT):
                        wc = wpool.tile([P, DIM], f32r, tag="w", name="wq_c")
                        nc.sync.dma_start(out=wc[:],
                                          in_=wq_in[ci * P:(ci + 1) * P, :])
                        for ot in range(CT):
                            nc.tensor.matmul(
                                qps[ot][:], wc[:, ot * P:(ot + 1) * P],
                                h_t[:, ci, :],
                                start=(ci == 0), stop=(ci == CT - 1))
                    q_t = []
                    for ot in range(CT):
                        qt_ = qpool.tile([P, NT], bf16, tag="q", name="q_t")
                        nc.scalar.activation(qt_[:], qps[ot][:], AF.Copy)
                        q_t.append(qt_)

            # ---- gathered K/V into SBUF ----
            k_g = []
            for hp in range(NPAIR):
                t = kgp.tile([P, GROUP, NT], bf16, tag="kg", name="k_g")
                src = cck_out[hp // 4].ap().bitcast(bf16).rearrange(
                    "(r c p) n -> p c r n", r=GROUP, p=P)
                nc.sync.dma_start(out=t[:], in_=src[:, hp % 4, :, :])
                k_g.append(t)
            # v_g split per collective chunk so attnV on heads 0-7 never
            # waits for the second V gather.
            v_g = [[], []]
            for half in range(2):
                for kt in range(KT):
                    t = vgp.tile([P, 8, DH + 1], bf16, tag=f"vg{half}",
                                 name="v_g", bufs=KT)
                    nc.gpsimd.memset(t[:, :, DH:DH + 1], 1.0)
                    r, tt = divmod(kt, GROUP)
                    src = ccv_out[half].ap().bitcast(bf16).rearrange(
                        "(r t p) (h d) -> p r t h d", r=GROUP, t=GROUP, h=8)
                    nc.sync.dma_start(out=t[:, :, 0:DH],
                                      in_=src[:, r, tt, :, :])
                    v_g[half].append(t)


            # =========================================================
            # Phase B: attention.
            #   scT[k, q] = K^T q  (per pair, two 64-contraction matmuls)
            #   at = exp(scT / 8)          (ScalarE, bf16)
            #   av[d(+den), q] += V_aug^T @ at   (v-stationary matmuls)
            #   ao[hd, q] = av[d, q] * recip(den)[q]
            # =========================================================
            ao_t = [aopool.tile([P, NT], f32r, tag="ao", name="ao")
                    for _ in range(CT)]
            with (
                tc.tile_pool(name="atp", bufs=8) as atp,
                tc.tile_pool(name="rcp", bufs=1) as rcp,
                tc.tile_pool(name="tmpb", bufs=2) as tmpbp,
                tc.tile_pool(name="scps", bufs=2, space="PSUM") as scps,
                tc.tile_pool(name="avps", bufs=3, space="PSUM") as avps,
                tc.tile_pool(name="bcps", bufs=1, space="PSUM") as bcps,
            ):
                prev = None  # (hp, av_pair) awaiting normalize

                def normalize(hp, av_pair):
                    for hi in range(2):
                        rc = rcp.tile([1, NT], bf16, tag="rc", name="rc", bufs=4)
                        with nc.allow_low_precision(
                                reason="fp22 recip of softmax denom is ample"):
                            nc.vector.reciprocal(rc[:],
                                                 av_pair[hi][DH:DH + 1, :])
                        bc = bcps.tile([DH, NT], f32, tag="bc", name="bc")
                        nc.tensor.matmul(bc[:], ones_row64[:],
                                         rc[:],
                                         start=True, stop=True)
                        bcs = tmpbp.tile([DH, NT], f32, tag="bcs",
                                         name="bcs", bufs=2)
                        nc.vector.tensor_copy(bcs[:], bc[:])
                        if hi == 0:
                            nc.vector.tensor_tensor(
                                ao_t[hp][0:DH, :], av_pair[0][0:DH, :],
                                bcs[:], ALU.mult)
                        else:
                            tmpb = tmpbp.tile([DH, NT], f32r, tag="tb",
                                              name="tmpb")
                            nc.vector.tensor_tensor(
                                tmpb[:], av_pair[1][0:DH, :], bcs[:],
                                ALU.mult)
                            nc.sync.dma_start(out=ao_t[hp][DH:P, :],
                                              in_=tmpb[:])

                def attnv(hp, kt, av, at_tile):
                    for hi in range(2):
                        hg = hp * 2 + hi
                        vg = v_g[hg // 8][kt]
                        nc.tensor.matmul(
                            av[hi][:],
                            vg[:, hg % 8, :],
                            at_tile[:, hi * NT:(hi + 1) * NT],
                            start=(kt == 0), stop=(kt == KT - 1))

                # attnV of pair p runs interleaved with the scores/exp of
                # pair p+1 (one-pair lag), so the Act engine is never the
                # PE's direct predecessor within a pair.
                at_store = {}
                for hp in range(NPAIR + 1):
                    if hp > 0:
                        av = [avps.tile([DH + 1, NT], f32, tag="av",
                                        name="av")
                              for _ in range(2)]
                    if prev is not None:
                        normalize(*prev)
                        prev = None
                    for kt in range(KT):
                        if hp < NPAIR:
                            sc = scps.tile([P, 2 * NT], f32, tag="sc",
                                           name="sc")
                            r, tt = divmod(kt, GROUP)
                            ksl = k_g[hp][:, r, tt * P:(tt + 1) * P]
                            nc.tensor.matmul(
                                sc[:, 0:NT],
                                ksl[0:DH, :],
                                q_t[hp][0:DH, :], start=True, stop=True)
                            nc.tensor.matmul(
                                sc[:, NT:2 * NT],
                                ksl[DH:P, :],
                                q_t[hp][DH:P, :], start=True, stop=True)
                            at = atp.tile([P, 2 * NT], bf16, tag="at",
                                          name="at", bufs=20)
                            nc.scalar.activation(
                                at[:], sc[:], AF.Exp,
                                scale=float(1.0 / np.sqrt(DH)))
                            at_store[(hp, kt)] = at
                        if hp > 0:
                            attnv(hp - 1, kt, av, at_store.pop((hp - 1, kt)))
                    if hp > 0:
                        prev = (hp - 1, av)
                normalize(*prev)

            if debug:
                for ci in range(CT):
                    nc.sync.dma_start(out=dbg_ao[ci * P:(ci + 1) * P, :],
                                      in_=ao_t[ci][:])

            # =========================================================
            # Phase C: output projection (wo streamed).
            # =========================================================
            with (
                tc.tile_pool(name="wop", bufs=3) as wop,
                tc.tile_pool(name="outsb", bufs=2) as outp,
                tc.tile_pool(name="ops", bufs=CT, space="PSUM") as ops,
            ):
                ops_t = [ops.tile([P, NT], f32, tag="o", name="ops_t")
                         for _ in range(CT)]
                for ci in range(CT):
                    wc = wop.tile([P, DIM], f32r, tag="wo", name="wo_c")
                    nc.sync.dma_start(out=wc[:],
                                      in_=wo_in[ci * P:(ci + 1) * P, :])
                    for ot in range(CT):
                        nc.tensor.matmul(
                            ops_t[ot][:], wc[:, ot * P:(ot + 1) * P],
                            ao_t[ci][:],
                            start=(ci == 0), stop=(ci == CT - 1))
                for ot in range(CT):
                    ost = outp.tile([P, NT], f32, tag="ou", name="ost")
                    nc.vector.tensor_copy(ost[:], ops_t[ot][:])
                    nc.sync.dma_start(out=out_ext[ot * P:(ot + 1) * P, :],
                                      in_=ost[:])

    nc.compile()
    return nc


def _get_nc(debug=False):
    key = ("nc", debug)
    if key not in _CACHE:
        _CACHE[key] = _build(debug)
    return _CACHE[key]


def kernel(x, w_qkv, w_out, ln_gamma, ln_beta, _profile=False, _debug=False):
    from concourse.bass_utils import run_bass_kernel_spmd

    x = np.asarray(x, np.float32)
    w_qkv = np.asarray(w_qkv, np.float32)
    w_out = np.asarray(w_out, np.float32)
    ln_gamma = np.asarray(ln_gamma, np.float32)
    ln_beta = np.asarray(ln_beta, np.float32)

    wq = np.ascontiguousarray(w_qkv[0:DIM].T)
    wk = np.ascontiguousarray(w_qkv[DIM:2 * DIM].T)
    wv = np.ascontiguousarray(w_qkv[2 * DIM:3 * DIM].T)
    wo = np.ascontiguousarray(w_out.T)
    grow = np.ascontiguousarray(ln_gamma.reshape(1, DIM))
    brow = np.ascontiguousarray(ln_beta.reshape(1, DIM))

    in_maps = []
    for j in range(NCORES):
        b, c = divmod(j, GROUP)
        in_maps.append({
            "x": np.ascontiguousarray(x[:, c * NT:(c + 1) * NT, b]),
            "wq": wq, "wk": wk, "wv": wv, "wo": wo,
            "gamma": grow, "beta": brow,
        })

    nc = _get_nc(_debug)
    res = run_bass_kernel_spmd(nc, in_maps, core_ids=list(range(NCORES)),
                               trace=_profile)
    if _profile:
        _CACHE["last_result"] = res

    out = np.empty((DIM, SEQ, BATCH), np.float32)
    for j in range(NCORES):
        b, c = divmod(j, GROUP)
        out[:, c * NT:(c + 1) * NT, b] = res.results[j]["out"]
    if _debug:
        _CACHE["dbg"] = res.results
    return out
